# revision 38
# baseline (speedup 1.0000x reference)
"""MinGRU block kernel for Trainium2 (Bass/Tile), SPMD over 8 NeuronCores.

Problem: B=8, S=2048, D=1024, F=3072 (nn_MinGRUBlock).
Sharding: data-parallel over batch (one batch row per core); weights replicated.

v2: all six matmul groups run in fp8(e4m3) with DoubleRow perf mode (2 fp8
weights per PE cell -> 2x MAC throughput), N=512 free dims. Weight tensors are
pre-scaled by power-of-2 factors into fp8 range; the inverse scales fold into
the (already present) ACT readout scale constants, so descaling is free.
Activations are quantized to fp8 with an 8x scale folded into the rmsnorm
reciprocal; the FFN z tile carries a 16x scale compensated at the final
residual readout (the residual scratch is written 131072x scaled via the
tensor_tensor_reduce output scale so the phase-2b add stays scale-consistent).

Per-core dataflow (compute in "T layout": feature on partitions, time free):
  phase 1 (mixer, s-chunks of 512, per-chunk stages):
    A: load x chunk, PE-transpose to xT, ACT squares, PE ones-reduce (norm1)
    B: sqrt/recip, GPSIMD partition-broadcast, xnT = xT*r -> fp8
    C: g/d/v projections as fp8 DoubleRow matmuls (4 MMs of K=256 each),
       ACT sigmoid/tanh readouts (tables batched per proj pass), DVE
       tensor_tensor_scan with fp32 carry, out1s = (x+h)*2^17, norm2 squares
    D: norm2 sqrt/recip/broadcast, o1n = out1s*r -> fp8 resident
  phase 2a: z = silu(gate)*up*16 in fp8 (gate via fused ACT Silu, z via
    DVE tensor_tensor_reduce reading the up PSUM directly)
  phase 2b: W_out DoubleRow matmuls + residual add + PE-transpose back,
    final 1/2^17 descale on the DVE copy out of transpose PSUM.
"""

import os
import sys
from contextlib import ExitStack

import numpy as np
import ml_dtypes

for _p in ("/opt/trn_rl_repo", "/root/.axon_site/_ro/trn_rl_repo"):
    if os.path.isdir(_p) and _p not in sys.path:
        sys.path.insert(0, _p)

import concourse.bass as bass
import concourse.tile as tile
from concourse import bacc, mybir
from concourse.bass_utils import run_bass_kernel_spmd

F32 = mybir.dt.float32
F16 = mybir.dt.float16
F8 = mybir.dt.float8e4
AF = mybir.ActivationFunctionType
OP = mybir.AluOpType
DR = mybir.MatmulPerfMode.DoubleRow

B, S, D, F = 8, 2048, 1024, 3072
EPS = 1e-6
KD = D // 128           # 8 d-ptiles
MD = D // 128           # 8
MFO = F // 128          # 24 f-ptiles
MF2 = 2 * F // 128      # 48 (gate|up)

CH = 512                # s-chunk (both phases)
NCH = S // CH           # 4
NST = CH // 128         # 4 s-tiles per chunk

# fp8 scaling constants (fixed powers of two; inputs are bounded by
# construction: |W{g,v,d}|<=1/32, |W_gate/up|<=1/32, |W_out|<=1/sqrt(3072))
AS = 8.0                # activation quantization scale (normalized acts)
S_MIX = 4096.0          # mixer weight scale      -> |w|*S <= 128
S_GU = 4096.0           # gate weight scale       -> |w|*S <= 128
S_UP = 4.0              # up weight scale (small so z = gate*ups fits fp8)
S_O = 8192.0            # out-proj weight scale   -> |w|*S <= 148
ZETA = AS * S_UP        # scale carried by the fp8 z tile (= 32)


def build_program():
    nc = bacc.Bacc("TRN2", target_bir_lowering=False, debug=False)

    x_d = nc.dram_tensor("x", [S, D], F32, kind="ExternalInput").ap()
    wmix_d = nc.dram_tensor("w_mix", [3 * MD, 128, KD, 128], F8, kind="ExternalInput").ap()
    bmix_d = nc.dram_tensor("b_mix", [128, 3 * MD], F32, kind="ExternalInput").ap()
    wgu_d = nc.dram_tensor("w_gu", [MF2, 128, KD, 128], F8, kind="ExternalInput").ap()
    wout_d = nc.dram_tensor("w_out", [MD, 128, MFO, 128], F8, kind="ExternalInput").ap()
    ident_d = nc.dram_tensor("ident", [128, 128], F32, kind="ExternalInput").ap()
    out_d = nc.dram_tensor("out", [S, D], F32, kind="ExternalOutput").ap()

    with tile.TileContext(nc) as tc, ExitStack() as top:
        # ---------- persistent tiles ----------
        cpool = top.enter_context(tc.tile_pool(name="consts", bufs=1))
        ident = cpool.tile([128, 128], F32)
        nc.sync.dma_start(ident[:], ident_d[:])
        ones_col = cpool.tile([128, 1], F16)
        nc.vector.memset(ones_col[:], 1.0)
        bmix = cpool.tile([128, 3 * MD], F32)
        nc.sync.dma_start(bmix[:], bmix_d[:])
        eps1 = cpool.tile([1, 1], F32)
        nc.vector.memset(eps1[:], EPS / (AS * AS))

        # DRAM scratch for the (scaled) mixer output residual, per chunk
        dpool = top.enter_context(tc.tile_pool(name="dscratch", bufs=1, space="DRAM"))
        sc1_t = [dpool.tile([KD, 128, CH], F32, name=f"sc1_{i}") for i in range(NCH)]

        # normalized out1 (x8) stays resident in SBUF across phase 1 -> 2a
        o1n_pool = top.enter_context(tc.tile_pool(name="o1n", bufs=1))
        o1n = o1n_pool.tile([128, KD, S], F8)

        carry_pool = top.enter_context(tc.tile_pool(name="carry", bufs=1))
        carry = carry_pool.tile([128, KD], F32)

        # pools that must survive into phase 2a (last chunk's norm2 is
        # emitted there so its latency chain hides under FFN matmuls)
        p_sq = top.enter_context(tc.tile_pool(name="sq", bufs=2))
        p_o1 = top.enter_context(tc.tile_pool(name="o1", bufs=2))
        p_row = top.enter_context(tc.tile_pool(name="rows", bufs=1))
        p_bc = top.enter_context(tc.tile_pool(name="bc", bufs=2))
        p_wgu = top.enter_context(tc.tile_pool(name="wgu", bufs=6))
        ps_ss = top.enter_context(tc.tile_pool(name="ss_ps", bufs=2, space="PSUM"))

        # ---------- phase 1: mixer ----------
        with ExitStack() as ph1:
            wpool = ph1.enter_context(tc.tile_pool(name="wmix", bufs=1))
            wmix = wpool.tile([128, 3 * MD, KD, 128], F8)
            wmix_dp = wmix_d.rearrange("m p k j -> p m k j")

            p_nat = ph1.enter_context(tc.tile_pool(name="xnat", bufs=5))
            p_xT = ph1.enter_context(tc.tile_pool(name="xT", bufs=2))
            p_x8 = ph1.enter_context(tc.tile_pool(name="x8", bufs=2))
            p_sg = ph1.enter_context(tc.tile_pool(name="sg", bufs=2))
            p_at = ph1.enter_context(tc.tile_pool(name="at", bufs=2))
            p_sm = ph1.enter_context(tc.tile_pool(name="sm", bufs=2))
            p_hT = ph1.enter_context(tc.tile_pool(name="hT", bufs=2))
            ps_tp = ph1.enter_context(tc.tile_pool(name="tp_ps", bufs=2, space="PSUM"))
            ps_mm = ph1.enter_context(tc.tile_pool(name="mm_ps", bufs=4, space="PSUM"))

            st = {}  # c -> dict of live tiles

            def stA(c):
                """load + transpose x chunk; norm1 squares + PE reduce."""
                s0 = c * CH
                d = st.setdefault(c, {})
                xT = p_xT.tile([128, KD, CH], F32, tag="xT", name=f"xT{c}")
                nats = []
                for stt in range(NST):
                    nat = p_nat.tile([128, D], F32, tag="nat", name=f"nat{c}_{stt}")
                    nc.sync.dma_start(nat[:], x_d[s0 + stt * 128: s0 + (stt + 1) * 128, :])
                    nats.append(nat)
                ss1 = ps_ss.tile([1, CH], F32, tag="ss", name=f"ss1_{c}")
                for kt in range(KD):
                    tp = ps_tp.tile([128, CH], F32, tag="tp", name=f"tp{c}_{kt}")
                    for stt in range(NST):
                        nc.tensor.transpose(tp[:, stt * 128:(stt + 1) * 128],
                                            nats[stt][:, kt * 128:(kt + 1) * 128],
                                            ident[:])
                    nc.vector.tensor_copy(xT[:, kt], tp[:])
                    sq = p_sq.tile([128, CH], F16, tag="sq1", name=f"sq1_{c}_{kt}")
                    nc.scalar.activation(sq[:], xT[:, kt], AF.Square, bias=0.0)
                    nc.tensor.matmul(ss1[:], ones_col[:], sq[:],
                                     start=(kt == 0), stop=(kt == KD - 1))
                d["xT"] = xT
                d["ss1"] = ss1

            def stB(c):
                """norm1 scale; xnT = AS * x / rms -> fp8."""
                d = st[c]
                srow = p_row.tile([1, CH], F32, tag="srow1", name=f"srow1_{c}")
                nc.scalar.activation(srow[:], d["ss1"][:], AF.Sqrt,
                                     bias=eps1[:], scale=1.0 / (AS * AS * D))
                rrow = p_row.tile([1, CH], F32, tag="rrow1", name=f"rrow1_{c}")
                nc.vector.reciprocal_approx_fast(rrow[:], srow[:])
                bc = p_bc.tile([128, CH], F32, tag="bc1", name=f"bc1_{c}")
                nc.gpsimd.partition_broadcast(bc[:], rrow[:])
                xnT = p_x8.tile([128, KD, CH], F8, tag="xnT", name=f"xnT{c}")
                for kt in range(KD):
                    nc.vector.tensor_tensor(xnT[:, kt], d["xT"][:, kt], bc[:], OP.mult)
                d["xnT"] = xnT

            def _proj(d, mt, out_ap, fn):
                ps = ps_mm.tile([128, CH], F32, tag="mm", name=f"mm_{mt}")
                for j in range(KD // 2):
                    nc.tensor.matmul(ps[:], wmix[:, mt, 2 * j:2 * j + 2, :],
                                     d["xnT"][:, 2 * j:2 * j + 2, :],
                                     start=(j == 0), stop=(j == KD // 2 - 1),
                                     perf_mode=DR)
                nc.scalar.activation(out_ap, ps[:], fn,
                                     bias=bmix[:, mt:mt + 1], scale=1.0 / (AS * S_MIX))

            def stC(c):
                """projections (fp8 DoubleRow), activations, scan, residual."""
                d = st[c]
                sg = p_sg.tile([128, KD, CH], F16, tag="sg", name=f"sg{c}")
                for kt in range(KD):          # g-pass (sigmoid table)
                    _proj(d, kt, sg[:, kt], AF.Sigmoid)
                a_t = p_at.tile([128, KD, CH], F16, tag="at", name=f"at{c}")
                for kt in range(KD):          # d-pass (sigmoid table)
                    sd = p_sm.tile([128, CH], F16, tag="sd", name=f"sd{c}_{kt}")
                    _proj(d, 2 * MD + kt, sd[:], AF.Sigmoid)
                    nc.vector.tensor_scalar(a_t[:, kt], sd[:], 0.998, 0.001,
                                            OP.mult, OP.add)
                out1 = p_o1.tile([128, KD, CH], F32, tag="o1", name=f"o1_{c}")
                for kt in range(KD):          # v-pass (tanh table) + scan chain
                    tv = p_sm.tile([128, CH], F16, tag="tv", name=f"tv{c}_{kt}")
                    _proj(d, MD + kt, tv[:], AF.Tanh)
                    xs = p_sm.tile([128, CH], F16, tag="xs", name=f"xs{c}_{kt}")
                    nc.vector.tensor_tensor(xs[:], sg[:, kt], tv[:], OP.mult)
                    hT = p_hT.tile([128, CH], F32, tag="hT", name=f"hT{c}_{kt}")
                    init = 0.0 if c == 0 else carry[:, kt:kt + 1]
                    nc.vector.tensor_tensor_scan(hT[:], a_t[:, kt], xs[:],
                                                 init, OP.mult, OP.add)
                    nc.vector.tensor_copy(carry[:, kt:kt + 1], hT[:, CH - 1:CH])
                    nc.vector.tensor_tensor(out1[:, kt], d["xT"][:, kt], hT[:],
                                            OP.add)
                    nc.sync.dma_start(sc1_t[c][kt], out1[:, kt])
                d["out1"] = out1
                if c < NCH - 1:
                    stNorm2(c)

            def stNorm2(c):
                """norm2 squares + PE reduce (deferred to 2a for the last
                chunk so the scan-chain drain hides under FFN matmuls)."""
                d = st[c]
                ss2 = ps_ss.tile([1, CH], F32, tag="ss", name=f"ss2_{c}")
                for kt in range(KD):          # (square table)
                    sq = p_sq.tile([128, CH], F16, tag="sq2", name=f"sq2_{c}_{kt}")
                    nc.scalar.activation(sq[:], d["out1"][:, kt], AF.Square,
                                         bias=0.0)
                    nc.tensor.matmul(ss2[:], ones_col[:], sq[:],
                                     start=(kt == 0), stop=(kt == KD - 1))
                d["ss2"] = ss2

            def stD(c):
                """norm2 scale; o1n = AS * out1 / rms -> fp8 resident."""
                d = st[c]
                s0 = c * CH
                srow = p_row.tile([1, CH], F32, tag="srow2", name=f"srow2_{c}")
                nc.scalar.activation(srow[:], d["ss2"][:], AF.Sqrt,
                                     bias=eps1[:], scale=1.0 / (AS * AS * D))
                rrow = p_row.tile([1, CH], F32, tag="rrow2", name=f"rrow2_{c}")
                nc.vector.reciprocal_approx_fast(rrow[:], srow[:])
                bc = p_bc.tile([128, CH], F32, tag="bc2", name=f"bc2_{c}")
                nc.gpsimd.partition_broadcast(bc[:], rrow[:])
                for kt in range(KD):
                    nc.vector.tensor_tensor(o1n[:, kt, s0:s0 + CH],
                                            d["out1"][:, kt], bc[:], OP.mult)
                del st[c]

            stA(0)
            # mixer weights per e-ptile so the first projections start early
            for mt in range(3 * MD):
                nc.sync.dma_start(wmix[:, mt], wmix_dp[:, mt])
            # prefetch the first two FFN weight pairs behind the mixer weights
            wgu_pre = {}
            for mg in range(2):
                wg = p_wgu.tile([128, KD, 128], F8, tag="wgu")
                nc.sync.dma_start(wg[:], wgu_d[mg])
                wu = p_wgu.tile([128, KD, 128], F8, tag="wgu")
                nc.sync.dma_start(wu[:], wgu_d[MFO + mg])
                wgu_pre[mg] = (wg, wu)
            stB(0)
            for c in range(NCH):
                if c + 1 < NCH:
                    stA(c + 1)
                stC(c)
                if c + 1 < NCH:
                    stB(c + 1)
                if c < NCH - 1:
                    stD(c)

        # ---------- phase 2: FFN ----------
        with ExitStack() as ph2:
            zpool = ph2.enter_context(tc.tile_pool(name="zbuf", bufs=1))
            z = zpool.tile([128, MFO, S], F8)
            wopool = ph2.enter_context(tc.tile_pool(name="wout", bufs=1))
            wout = wopool.tile([128, MD, MFO, 128], F8)
            nc.sync.dma_start(wout[:], wout_d.rearrange("m p k j -> p m k j"))

            # 2a: gate/up + z
            with ExitStack() as ph2a:
                p_gt = ph2a.enter_context(tc.tile_pool(name="gt", bufs=3))
                ps_gu = ph2a.enter_context(tc.tile_pool(name="gu_ps", bufs=6, space="PSUM"))

                def gu_group(mg, sc, wg, wu):
                    sl = slice(sc * CH, (sc + 1) * CH)
                    gps = ps_gu.tile([128, CH], F32, tag="gups")
                    for j in range(KD // 2):
                        nc.tensor.matmul(gps[:], wg[:, 2 * j:2 * j + 2, :],
                                         o1n[:, 2 * j:2 * j + 2, sl],
                                         start=(j == 0), stop=(j == KD // 2 - 1),
                                         perf_mode=DR)
                    ups = ps_gu.tile([128, CH], F32, tag="gups")
                    for j in range(KD // 2):
                        nc.tensor.matmul(ups[:], wu[:, 2 * j:2 * j + 2, :],
                                         o1n[:, 2 * j:2 * j + 2, sl],
                                         start=(j == 0), stop=(j == KD // 2 - 1),
                                         perf_mode=DR)
                    gate = p_gt.tile([128, CH], F16, tag="gate")
                    nc.scalar.activation(gate[:], gps[:], AF.Silu,
                                         bias=0.0, scale=1.0 / (AS * S_GU))
                    # z = silu(G) * (AS*S_UP*U): fp8 tile carries ZETA=32
                    nc.vector.tensor_tensor(z[:, mg, sl], gate[:], ups[:],
                                            OP.mult)

                # chunks 0-2 of the first two f-tiles run while the last
                # mixer chunk's norm2/scan chain drains; then its sc=3 slabs
                for mg in range(2):
                    for sc in range(NCH - 1):
                        gu_group(mg, sc, *wgu_pre[mg])
                stNorm2(NCH - 1)
                stD(NCH - 1)
                for mg in range(2):
                    gu_group(mg, NCH - 1, *wgu_pre[mg])
                for mg in range(2, MFO):
                    wg = p_wgu.tile([128, KD, 128], F8, tag="wgu")
                    nc.sync.dma_start(wg[:], wgu_d[mg])
                    wu = p_wgu.tile([128, KD, 128], F8, tag="wgu")
                    nc.sync.dma_start(wu[:], wgu_d[MFO + mg])
                    for sc in range(NCH):
                        gu_group(mg, sc, wg, wu)

            # 2b: W_out + residual + transpose out
            with ExitStack() as ph2b:
                p_o1c = ph2b.enter_context(tc.tile_pool(name="o1c", bufs=3))
                p_yy = ph2b.enter_context(tc.tile_pool(name="yy", bufs=2))
                p_oT = ph2b.enter_context(tc.tile_pool(name="outT", bufs=MD + 1))
                p_onat = ph2b.enter_context(tc.tile_pool(name="onat", bufs=3))
                ps_y = ph2b.enter_context(tc.tile_pool(name="y_ps", bufs=3, space="PSUM"))
                ps_t2 = ph2b.enter_context(tc.tile_pool(name="t2_ps", bufs=3, space="PSUM"))
                for sc in range(NCH):
                    sl = slice(sc * CH, (sc + 1) * CH)
                    outTs = []
                    for mo in range(MD):
                        yps = ps_y.tile([128, CH], F32, tag="yps")
                        for j in range(MFO // 2):
                            nc.tensor.matmul(yps[:], wout[:, mo, 2 * j:2 * j + 2, :],
                                             z[:, 2 * j:2 * j + 2, sl],
                                             start=(j == 0), stop=(j == MFO // 2 - 1),
                                             perf_mode=DR)
                        o1c = p_o1c.tile([128, CH], F32, tag="o1c")
                        nc.sync.dma_start(o1c[:], sc1_t[sc][mo])
                        yy = p_yy.tile([128, CH], F32, tag="yy")
                        nc.vector.tensor_scalar(yy[:], yps[:], 1.0 / (ZETA * S_O),
                                                0.0, OP.mult, OP.add)
                        oT = p_oT.tile([128, CH], F32, tag="oT")
                        nc.vector.tensor_tensor(oT[:], yy[:], o1c[:], OP.add)
                        outTs.append(oT)
                    for q in range(NST):
                        onat = p_onat.tile([128, D], F32, tag="onat")
                        for h in range(2):
                            t2 = ps_t2.tile([128, 512], F32, tag="t2")
                            for jj in range(4):
                                nc.tensor.transpose(
                                    t2[:, jj * 128:(jj + 1) * 128],
                                    outTs[4 * h + jj][:, q * 128:(q + 1) * 128],
                                    ident[:])
                            nc.vector.tensor_copy(onat[:, h * 512:(h + 1) * 512],
                                                  t2[:])
                        srow0 = sc * CH + q * 128
                        nc.sync.dma_start(out_d[srow0:srow0 + 128, :], onat[:])

    nc.compile()
    return nc


_NC = None


def _get_nc():
    global _NC
    if _NC is None:
        _NC = build_program()
    return _NC


def _q8(a, s):
    return np.clip(np.asarray(a, np.float32) * s, -240.0, 240.0).astype(
        ml_dtypes.float8_e4m3)


def _prep_weights(inputs):
    w1 = np.asarray(inputs["rms_mix_w"], np.float32)
    w2 = np.asarray(inputs["rms_ffn_w"], np.float32)
    Wg = np.asarray(inputs["Wg"], np.float32) * w1[None, :]
    Wv = np.asarray(inputs["Wv"], np.float32) * w1[None, :]
    Wd = np.asarray(inputs["Wd"], np.float32) * w1[None, :]
    Wcat = np.concatenate([Wg, Wv, Wd], axis=0)            # [3D, D]
    w_mix = _q8(np.ascontiguousarray(
        Wcat.T.reshape(KD, 128, 3 * MD, 128).transpose(2, 1, 0, 3)), S_MIX)
    bcat = np.concatenate([np.asarray(inputs["bg"], np.float32),
                           np.asarray(inputs["bv"], np.float32),
                           np.asarray(inputs["bd"], np.float32)])
    b_mix = np.ascontiguousarray(bcat.reshape(3 * MD, 128).T).astype(np.float32)
    Wgate = np.asarray(inputs["W_gate"], np.float32) * w2[None, :]
    Wup = np.asarray(inputs["W_up"], np.float32) * w2[None, :]
    Wcat2 = np.concatenate([Wgate * S_GU, Wup * S_UP], axis=0)  # [2F, D], pre-scaled
    w_gu = _q8(np.ascontiguousarray(
        Wcat2.T.reshape(KD, 128, MF2, 128).transpose(2, 1, 0, 3)), 1.0)
    WoT = np.asarray(inputs["W_out"], np.float32).T        # [F, D]
    w_out = _q8(np.ascontiguousarray(
        WoT.reshape(MFO, 128, MD, 128).transpose(2, 1, 0, 3)), S_O)
    return {
        "w_mix": w_mix, "b_mix": b_mix, "w_gu": w_gu, "w_out": w_out,
        "ident": np.eye(128, dtype=np.float32),
    }


def run(inputs, trace=False, **kw):
    x = np.asarray(inputs["x"], np.float32)
    shared = _prep_weights(inputs)
    in_maps = [dict(shared, x=np.ascontiguousarray(x[b])) for b in range(B)]
    res = run_bass_kernel_spmd(_get_nc(), in_maps, list(range(B)), trace=trace, **kw)
    out = np.stack([np.asarray(res.results[b]["out"], np.float32) for b in range(B)])
    return out, res


def kernel(**inputs) -> np.ndarray:
    out, _ = run(inputs)
    return out


# revision 39
# speedup vs baseline: 1.0129x; 1.0129x over previous
"""MinGRU block kernel for Trainium2 (Bass/Tile), SPMD over 8 NeuronCores.

Problem: B=8, S=2048, D=1024, F=3072 (nn_MinGRUBlock).
Sharding: data-parallel over batch (one batch row per core); weights replicated.

v2: all six matmul groups run in fp8(e4m3) with DoubleRow perf mode (2 fp8
weights per PE cell -> 2x MAC throughput), N=512 free dims. Weight tensors are
pre-scaled by power-of-2 factors into fp8 range; the inverse scales fold into
the (already present) ACT readout scale constants, so descaling is free.
Activations are quantized to fp8 with an 8x scale folded into the rmsnorm
reciprocal; the FFN z tile carries a 16x scale compensated at the final
residual readout (the residual scratch is written 131072x scaled via the
tensor_tensor_reduce output scale so the phase-2b add stays scale-consistent).

Per-core dataflow (compute in "T layout": feature on partitions, time free):
  phase 1 (mixer, s-chunks of 512, per-chunk stages):
    A: load x chunk, PE-transpose to xT, ACT squares, PE ones-reduce (norm1)
    B: sqrt/recip, GPSIMD partition-broadcast, xnT = xT*r -> fp8
    C: g/d/v projections as fp8 DoubleRow matmuls (4 MMs of K=256 each),
       ACT sigmoid/tanh readouts (tables batched per proj pass), DVE
       tensor_tensor_scan with fp32 carry, out1s = (x+h)*2^17, norm2 squares
    D: norm2 sqrt/recip/broadcast, o1n = out1s*r -> fp8 resident
  phase 2a: z = silu(gate)*up*16 in fp8 (gate via fused ACT Silu, z via
    DVE tensor_tensor_reduce reading the up PSUM directly)
  phase 2b: W_out DoubleRow matmuls + residual add + PE-transpose back,
    final 1/2^17 descale on the DVE copy out of transpose PSUM.
"""

import os
import sys
from contextlib import ExitStack

import numpy as np
import ml_dtypes

for _p in ("/opt/trn_rl_repo", "/root/.axon_site/_ro/trn_rl_repo"):
    if os.path.isdir(_p) and _p not in sys.path:
        sys.path.insert(0, _p)

import concourse.bass as bass
import concourse.tile as tile
from concourse import bacc, mybir
from concourse.bass_utils import run_bass_kernel_spmd

F32 = mybir.dt.float32
F16 = mybir.dt.float16
F8 = mybir.dt.float8e4
AF = mybir.ActivationFunctionType
OP = mybir.AluOpType
DR = mybir.MatmulPerfMode.DoubleRow

B, S, D, F = 8, 2048, 1024, 3072
EPS = 1e-6
KD = D // 128           # 8 d-ptiles
MD = D // 128           # 8
MFO = F // 128          # 24 f-ptiles
MF2 = 2 * F // 128      # 48 (gate|up)

CH = 512                # s-chunk (both phases)
NCH = S // CH           # 4
NST = CH // 128         # 4 s-tiles per chunk

# fp8 scaling constants (fixed powers of two; inputs are bounded by
# construction: |W{g,v,d}|<=1/32, |W_gate/up|<=1/32, |W_out|<=1/sqrt(3072))
AS = 8.0                # activation quantization scale (normalized acts)
S_MIX = 4096.0          # mixer weight scale      -> |w|*S <= 128
S_GU = 4096.0           # gate weight scale       -> |w|*S <= 128
S_UP = 4.0              # up weight scale (small so z = gate*ups fits fp8)
S_O = 8192.0            # out-proj weight scale   -> |w|*S <= 148
ZETA = AS * S_UP        # scale carried by the fp8 z tile (= 32)


def build_program():
    nc = bacc.Bacc("TRN2", target_bir_lowering=False, debug=False)

    x_d = nc.dram_tensor("x", [S, D], F32, kind="ExternalInput").ap()
    wmix_d = nc.dram_tensor("w_mix", [3 * MD, 128, KD, 128], F8, kind="ExternalInput").ap()
    bmix_d = nc.dram_tensor("b_mix", [128, 3 * MD], F32, kind="ExternalInput").ap()
    wgu_d = nc.dram_tensor("w_gu", [MF2, 128, KD, 128], F8, kind="ExternalInput").ap()
    wout_d = nc.dram_tensor("w_out", [MD, 128, MFO, 128], F8, kind="ExternalInput").ap()
    ident_d = nc.dram_tensor("ident", [128, 128], F32, kind="ExternalInput").ap()
    out_d = nc.dram_tensor("out", [S, D], F32, kind="ExternalOutput").ap()

    with tile.TileContext(nc) as tc, ExitStack() as top:
        # ---------- persistent tiles ----------
        cpool = top.enter_context(tc.tile_pool(name="consts", bufs=1))
        ident = cpool.tile([128, 128], F32)
        nc.sync.dma_start(ident[:], ident_d[:])
        ones_col = cpool.tile([128, 1], F16)
        nc.vector.memset(ones_col[:], 1.0)
        bmix = cpool.tile([128, 3 * MD], F32)
        nc.sync.dma_start(bmix[:], bmix_d[:])
        eps1 = cpool.tile([1, 1], F32)
        nc.vector.memset(eps1[:], EPS / (AS * AS))

        # DRAM scratch for the (scaled) mixer output residual, per chunk
        dpool = top.enter_context(tc.tile_pool(name="dscratch", bufs=1, space="DRAM"))
        sc1_t = [dpool.tile([KD, 128, CH], F32, name=f"sc1_{i}") for i in range(NCH)]

        # normalized out1 (x8) stays resident in SBUF across phase 1 -> 2a
        o1n_pool = top.enter_context(tc.tile_pool(name="o1n", bufs=1))
        o1n = o1n_pool.tile([128, KD, S], F8)

        carry_pool = top.enter_context(tc.tile_pool(name="carry", bufs=1))
        carry = carry_pool.tile([128, KD], F32)

        # pools that must survive into phase 2a (last chunk's norm2 is
        # emitted there so its latency chain hides under FFN matmuls)
        p_sq = top.enter_context(tc.tile_pool(name="sq", bufs=2))
        p_o1 = top.enter_context(tc.tile_pool(name="o1", bufs=2))
        p_row = top.enter_context(tc.tile_pool(name="rows", bufs=1))
        p_bc = top.enter_context(tc.tile_pool(name="bc", bufs=2))
        p_wgu = top.enter_context(tc.tile_pool(name="wgu", bufs=6))
        ps_ss = top.enter_context(tc.tile_pool(name="ss_ps", bufs=2, space="PSUM"))

        # ---------- phase 1: mixer ----------
        with ExitStack() as ph1:
            wpool = ph1.enter_context(tc.tile_pool(name="wmix", bufs=1))
            wmix = wpool.tile([128, 3 * MD, KD, 128], F8)
            wmix_dp = wmix_d.rearrange("m p k j -> p m k j")

            p_nat = ph1.enter_context(tc.tile_pool(name="xnat", bufs=5))
            p_xT = ph1.enter_context(tc.tile_pool(name="xT", bufs=2))
            p_x8 = ph1.enter_context(tc.tile_pool(name="x8", bufs=2))
            p_sg = ph1.enter_context(tc.tile_pool(name="sg", bufs=2))
            p_at = ph1.enter_context(tc.tile_pool(name="at", bufs=2))
            p_sm = ph1.enter_context(tc.tile_pool(name="sm", bufs=2))
            p_hT = ph1.enter_context(tc.tile_pool(name="hT", bufs=2))
            ps_tp = ph1.enter_context(tc.tile_pool(name="tp_ps", bufs=2, space="PSUM"))
            ps_mm = ph1.enter_context(tc.tile_pool(name="mm_ps", bufs=4, space="PSUM"))

            st = {}  # c -> dict of live tiles

            def stA(c):
                """load + transpose x chunk; norm1 squares + PE reduce."""
                s0 = c * CH
                d = st.setdefault(c, {})
                xT = p_xT.tile([128, KD, CH], F32, tag="xT", name=f"xT{c}")
                nats = []
                for stt in range(NST):
                    nat = p_nat.tile([128, D], F32, tag="nat", name=f"nat{c}_{stt}")
                    nc.sync.dma_start(nat[:], x_d[s0 + stt * 128: s0 + (stt + 1) * 128, :])
                    nats.append(nat)
                ss1 = ps_ss.tile([1, CH], F32, tag="ss", name=f"ss1_{c}")
                for kt in range(KD):
                    tp = ps_tp.tile([128, CH], F32, tag="tp", name=f"tp{c}_{kt}")
                    for stt in range(NST):
                        nc.tensor.transpose(tp[:, stt * 128:(stt + 1) * 128],
                                            nats[stt][:, kt * 128:(kt + 1) * 128],
                                            ident[:])
                    nc.vector.tensor_copy(xT[:, kt], tp[:])
                    sq = p_sq.tile([128, CH], F16, tag="sq1", name=f"sq1_{c}_{kt}")
                    nc.scalar.activation(sq[:], xT[:, kt], AF.Square, bias=0.0)
                    nc.tensor.matmul(ss1[:], ones_col[:], sq[:],
                                     start=(kt == 0), stop=(kt == KD - 1))
                d["xT"] = xT
                d["ss1"] = ss1

            def stB(c):
                """norm1 scale; xnT = AS * x / rms -> fp8."""
                d = st[c]
                srow = p_row.tile([1, CH], F32, tag="srow1", name=f"srow1_{c}")
                nc.scalar.activation(srow[:], d["ss1"][:], AF.Sqrt,
                                     bias=eps1[:], scale=1.0 / (AS * AS * D))
                rrow = p_row.tile([1, CH], F32, tag="rrow1", name=f"rrow1_{c}")
                nc.vector.reciprocal_approx_fast(rrow[:], srow[:])
                bc = p_bc.tile([128, CH], F32, tag="bc1", name=f"bc1_{c}")
                nc.gpsimd.partition_broadcast(bc[:], rrow[:])
                xnT = p_x8.tile([128, KD, CH], F8, tag="xnT", name=f"xnT{c}")
                for kt in range(KD):
                    nc.vector.tensor_tensor(xnT[:, kt], d["xT"][:, kt], bc[:], OP.mult)
                d["xnT"] = xnT

            def _proj(d, mt, out_ap, fn):
                ps = ps_mm.tile([128, CH], F32, tag="mm", name=f"mm_{mt}")
                for j in range(KD // 2):
                    nc.tensor.matmul(ps[:], wmix[:, mt, 2 * j:2 * j + 2, :],
                                     d["xnT"][:, 2 * j:2 * j + 2, :],
                                     start=(j == 0), stop=(j == KD // 2 - 1),
                                     perf_mode=DR)
                nc.scalar.activation(out_ap, ps[:], fn,
                                     bias=bmix[:, mt:mt + 1], scale=1.0 / (AS * S_MIX))

            def stC(c):
                """projections (fp8 DoubleRow), activations, scan, residual."""
                d = st[c]
                sg = p_sg.tile([128, KD, CH], F16, tag="sg", name=f"sg{c}")
                for kt in range(KD):          # g-pass (sigmoid table)
                    _proj(d, kt, sg[:, kt], AF.Sigmoid)
                a_t = p_at.tile([128, KD, CH], F16, tag="at", name=f"at{c}")
                for kt in range(KD):          # d-pass (sigmoid table)
                    sd = p_sm.tile([128, CH], F16, tag="sd", name=f"sd{c}_{kt}")
                    _proj(d, 2 * MD + kt, sd[:], AF.Sigmoid)
                    nc.vector.tensor_scalar(a_t[:, kt], sd[:], 0.998, 0.001,
                                            OP.mult, OP.add)
                out1 = p_o1.tile([128, KD, CH], F32, tag="o1", name=f"o1_{c}")
                for kt in range(KD):          # v-pass (tanh table) + scan chain
                    tv = p_sm.tile([128, CH], F16, tag="tv", name=f"tv{c}_{kt}")
                    _proj(d, MD + kt, tv[:], AF.Tanh)
                    xs = p_sm.tile([128, CH], F16, tag="xs", name=f"xs{c}_{kt}")
                    nc.vector.tensor_tensor(xs[:], sg[:, kt], tv[:], OP.mult)
                    hT = p_hT.tile([128, CH], F32, tag="hT", name=f"hT{c}_{kt}")
                    init = 0.0 if c == 0 else carry[:, kt:kt + 1]
                    nc.vector.tensor_tensor_scan(hT[:], a_t[:, kt], xs[:],
                                                 init, OP.mult, OP.add)
                    nc.vector.tensor_copy(carry[:, kt:kt + 1], hT[:, CH - 1:CH])
                    nc.vector.tensor_tensor(out1[:, kt], d["xT"][:, kt], hT[:],
                                            OP.add)
                    nc.sync.dma_start(sc1_t[c][kt], out1[:, kt])
                d["out1"] = out1
                if c < NCH - 1:
                    stNorm2(c)

            def stNorm2(c):
                """norm2 squares + PE reduce (deferred to 2a for the last
                chunk so the scan-chain drain hides under FFN matmuls)."""
                d = st[c]
                ss2 = ps_ss.tile([1, CH], F32, tag="ss", name=f"ss2_{c}")
                for kt in range(KD):          # (square table)
                    sq = p_sq.tile([128, CH], F16, tag="sq2", name=f"sq2_{c}_{kt}")
                    nc.scalar.activation(sq[:], d["out1"][:, kt], AF.Square,
                                         bias=0.0)
                    nc.tensor.matmul(ss2[:], ones_col[:], sq[:],
                                     start=(kt == 0), stop=(kt == KD - 1))
                d["ss2"] = ss2

            def stD(c):
                """norm2 scale; o1n = AS * out1 / rms -> fp8 resident."""
                d = st[c]
                s0 = c * CH
                srow = p_row.tile([1, CH], F32, tag="srow2", name=f"srow2_{c}")
                nc.scalar.activation(srow[:], d["ss2"][:], AF.Sqrt,
                                     bias=eps1[:], scale=1.0 / (AS * AS * D))
                rrow = p_row.tile([1, CH], F32, tag="rrow2", name=f"rrow2_{c}")
                nc.vector.reciprocal_approx_fast(rrow[:], srow[:])
                bc = p_bc.tile([128, CH], F32, tag="bc2", name=f"bc2_{c}")
                nc.gpsimd.partition_broadcast(bc[:], rrow[:])
                for kt in range(KD):
                    nc.vector.tensor_tensor(o1n[:, kt, s0:s0 + CH],
                                            d["out1"][:, kt], bc[:], OP.mult)
                del st[c]

            stA(0)
            # mixer weights per e-ptile so the first projections start early
            for mt in range(3 * MD):
                nc.sync.dma_start(wmix[:, mt], wmix_dp[:, mt])
            # prefetch the first two FFN weight pairs behind the mixer weights
            wgu_pre = {}
            for mg in range(2):
                wg = p_wgu.tile([128, KD, 128], F8, tag="wgu")
                nc.sync.dma_start(wg[:], wgu_d[mg])
                wu = p_wgu.tile([128, KD, 128], F8, tag="wgu")
                nc.sync.dma_start(wu[:], wgu_d[MFO + mg])
                wgu_pre[mg] = (wg, wu)
            stB(0)
            for c in range(NCH):
                if c + 1 < NCH:
                    stA(c + 1)
                stC(c)
                if c + 1 < NCH:
                    stB(c + 1)
                if c < NCH - 1:
                    stD(c)

        # ---------- phase 2: FFN ----------
        with ExitStack() as ph2:
            zpool = ph2.enter_context(tc.tile_pool(name="zbuf", bufs=1))
            z = zpool.tile([128, MFO, S], F8)
            wopool = ph2.enter_context(tc.tile_pool(name="wout", bufs=1))
            wout = wopool.tile([128, MD, MFO, 128], F8)
            nc.sync.dma_start(wout[:], wout_d.rearrange("m p k j -> p m k j"))

            # 2a: gate/up + z
            with ExitStack() as ph2a:
                p_gt = ph2a.enter_context(tc.tile_pool(name="gt", bufs=3))
                ps_gu = ph2a.enter_context(tc.tile_pool(name="gu_ps", bufs=4, space="PSUM"))

                def gu_group(mg, sc, wg, wu):
                    sl = slice(sc * CH, (sc + 1) * CH)
                    gps = ps_gu.tile([128, CH], F32, tag="gups")
                    for j in range(KD // 2):
                        nc.tensor.matmul(gps[:], wg[:, 2 * j:2 * j + 2, :],
                                         o1n[:, 2 * j:2 * j + 2, sl],
                                         start=(j == 0), stop=(j == KD // 2 - 1),
                                         perf_mode=DR)
                    ups = ps_gu.tile([128, CH], F32, tag="gups")
                    for j in range(KD // 2):
                        nc.tensor.matmul(ups[:], wu[:, 2 * j:2 * j + 2, :],
                                         o1n[:, 2 * j:2 * j + 2, sl],
                                         start=(j == 0), stop=(j == KD // 2 - 1),
                                         perf_mode=DR)
                    gate = p_gt.tile([128, CH], F16, tag="gate")
                    nc.scalar.activation(gate[:], gps[:], AF.Silu,
                                         bias=0.0, scale=1.0 / (AS * S_GU))
                    # z = silu(G) * (AS*S_UP*U): fp8 tile carries ZETA=32
                    nc.vector.tensor_tensor(z[:, mg, sl], gate[:], ups[:],
                                            OP.mult)

                # chunks 0-2 of the first two f-tiles run while the last
                # mixer chunk's norm2/scan chain drains; then its sc=3 slabs
                for mg in range(2):
                    for sc in range(NCH - 1):
                        gu_group(mg, sc, *wgu_pre[mg])
                stNorm2(NCH - 1)
                stD(NCH - 1)
                for mg in range(2):
                    gu_group(mg, NCH - 1, *wgu_pre[mg])
                for mg in range(2, MFO):
                    wg = p_wgu.tile([128, KD, 128], F8, tag="wgu")
                    nc.sync.dma_start(wg[:], wgu_d[mg])
                    wu = p_wgu.tile([128, KD, 128], F8, tag="wgu")
                    nc.sync.dma_start(wu[:], wgu_d[MFO + mg])
                    for sc in range(NCH):
                        gu_group(mg, sc, wg, wu)

            # 2b: W_out + residual + transpose out
            with ExitStack() as ph2b:
                p_o1c = ph2b.enter_context(tc.tile_pool(name="o1c", bufs=3))
                p_yy = ph2b.enter_context(tc.tile_pool(name="yy", bufs=2))
                p_oT = ph2b.enter_context(tc.tile_pool(name="outT", bufs=MD + 1))
                p_onat = ph2b.enter_context(tc.tile_pool(name="onat", bufs=3))
                ps_y = ph2b.enter_context(tc.tile_pool(name="y_ps", bufs=2, space="PSUM"))
                ps_t2 = ph2b.enter_context(tc.tile_pool(name="t2_ps", bufs=2, space="PSUM"))
                for sc in range(NCH):
                    sl = slice(sc * CH, (sc + 1) * CH)
                    outTs = []
                    for mo in range(MD):
                        yps = ps_y.tile([128, CH], F32, tag="yps")
                        for j in range(MFO // 2):
                            nc.tensor.matmul(yps[:], wout[:, mo, 2 * j:2 * j + 2, :],
                                             z[:, 2 * j:2 * j + 2, sl],
                                             start=(j == 0), stop=(j == MFO // 2 - 1),
                                             perf_mode=DR)
                        o1c = p_o1c.tile([128, CH], F32, tag="o1c")
                        nc.sync.dma_start(o1c[:], sc1_t[sc][mo])
                        yy = p_yy.tile([128, CH], F32, tag="yy")
                        nc.vector.tensor_scalar(yy[:], yps[:], 1.0 / (ZETA * S_O),
                                                0.0, OP.mult, OP.add)
                        oT = p_oT.tile([128, CH], F32, tag="oT")
                        nc.vector.tensor_tensor(oT[:], yy[:], o1c[:], OP.add)
                        outTs.append(oT)
                    for q in range(NST):
                        onat = p_onat.tile([128, D], F32, tag="onat")
                        for h in range(2):
                            t2 = ps_t2.tile([128, 512], F32, tag="t2")
                            for jj in range(4):
                                nc.tensor.transpose(
                                    t2[:, jj * 128:(jj + 1) * 128],
                                    outTs[4 * h + jj][:, q * 128:(q + 1) * 128],
                                    ident[:])
                            nc.vector.tensor_copy(onat[:, h * 512:(h + 1) * 512],
                                                  t2[:])
                        srow0 = sc * CH + q * 128
                        nc.sync.dma_start(out_d[srow0:srow0 + 128, :], onat[:])

    nc.compile()
    return nc


_NC = None


def _get_nc():
    global _NC
    if _NC is None:
        _NC = build_program()
    return _NC


def _q8(a, s):
    return np.clip(np.asarray(a, np.float32) * s, -240.0, 240.0).astype(
        ml_dtypes.float8_e4m3)


def _prep_weights(inputs):
    w1 = np.asarray(inputs["rms_mix_w"], np.float32)
    w2 = np.asarray(inputs["rms_ffn_w"], np.float32)
    Wg = np.asarray(inputs["Wg"], np.float32) * w1[None, :]
    Wv = np.asarray(inputs["Wv"], np.float32) * w1[None, :]
    Wd = np.asarray(inputs["Wd"], np.float32) * w1[None, :]
    Wcat = np.concatenate([Wg, Wv, Wd], axis=0)            # [3D, D]
    w_mix = _q8(np.ascontiguousarray(
        Wcat.T.reshape(KD, 128, 3 * MD, 128).transpose(2, 1, 0, 3)), S_MIX)
    bcat = np.concatenate([np.asarray(inputs["bg"], np.float32),
                           np.asarray(inputs["bv"], np.float32),
                           np.asarray(inputs["bd"], np.float32)])
    b_mix = np.ascontiguousarray(bcat.reshape(3 * MD, 128).T).astype(np.float32)
    Wgate = np.asarray(inputs["W_gate"], np.float32) * w2[None, :]
    Wup = np.asarray(inputs["W_up"], np.float32) * w2[None, :]
    Wcat2 = np.concatenate([Wgate * S_GU, Wup * S_UP], axis=0)  # [2F, D], pre-scaled
    w_gu = _q8(np.ascontiguousarray(
        Wcat2.T.reshape(KD, 128, MF2, 128).transpose(2, 1, 0, 3)), 1.0)
    WoT = np.asarray(inputs["W_out"], np.float32).T        # [F, D]
    w_out = _q8(np.ascontiguousarray(
        WoT.reshape(MFO, 128, MD, 128).transpose(2, 1, 0, 3)), S_O)
    return {
        "w_mix": w_mix, "b_mix": b_mix, "w_gu": w_gu, "w_out": w_out,
        "ident": np.eye(128, dtype=np.float32),
    }


def run(inputs, trace=False, **kw):
    x = np.asarray(inputs["x"], np.float32)
    shared = _prep_weights(inputs)
    in_maps = [dict(shared, x=np.ascontiguousarray(x[b])) for b in range(B)]
    res = run_bass_kernel_spmd(_get_nc(), in_maps, list(range(B)), trace=trace, **kw)
    out = np.stack([np.asarray(res.results[b]["out"], np.float32) for b in range(B)])
    return out, res


def kernel(**inputs) -> np.ndarray:
    out, _ = run(inputs)
    return out


# revision 42
# speedup vs baseline: 1.0280x; 1.0149x over previous
"""MinGRU block kernel for Trainium2 (Bass/Tile), SPMD over 8 NeuronCores.

Problem: B=8, S=2048, D=1024, F=3072 (nn_MinGRUBlock).
Sharding: data-parallel over batch (one batch row per core); weights replicated.

v2: all six matmul groups run in fp8(e4m3) with DoubleRow perf mode (2 fp8
weights per PE cell -> 2x MAC throughput), N=512 free dims. Weight tensors are
pre-scaled by power-of-2 factors into fp8 range; the inverse scales fold into
the (already present) ACT readout scale constants, so descaling is free.
Activations are quantized to fp8 with an 8x scale folded into the rmsnorm
reciprocal; the FFN z tile carries a 16x scale compensated at the final
residual readout (the residual scratch is written 131072x scaled via the
tensor_tensor_reduce output scale so the phase-2b add stays scale-consistent).

Per-core dataflow (compute in "T layout": feature on partitions, time free):
  phase 1 (mixer, s-chunks of 512, per-chunk stages):
    A: load x chunk, PE-transpose to xT, ACT squares, PE ones-reduce (norm1)
    B: sqrt/recip, GPSIMD partition-broadcast, xnT = xT*r -> fp8
    C: g/d/v projections as fp8 DoubleRow matmuls (4 MMs of K=256 each),
       ACT sigmoid/tanh readouts (tables batched per proj pass), DVE
       tensor_tensor_scan with fp32 carry, out1s = (x+h)*2^17, norm2 squares
    D: norm2 sqrt/recip/broadcast, o1n = out1s*r -> fp8 resident
  phase 2a: z = silu(gate)*up*16 in fp8 (gate via fused ACT Silu, z via
    DVE tensor_tensor_reduce reading the up PSUM directly)
  phase 2b: W_out DoubleRow matmuls + residual add + PE-transpose back,
    final 1/2^17 descale on the DVE copy out of transpose PSUM.
"""

import os
import sys
from contextlib import ExitStack

import numpy as np
import ml_dtypes

for _p in ("/opt/trn_rl_repo", "/root/.axon_site/_ro/trn_rl_repo"):
    if os.path.isdir(_p) and _p not in sys.path:
        sys.path.insert(0, _p)

import concourse.bass as bass
import concourse.tile as tile
from concourse import bacc, mybir
from concourse.bass_utils import run_bass_kernel_spmd

F32 = mybir.dt.float32
F32R = mybir.dt.float32r
F16 = mybir.dt.float16
F8 = mybir.dt.float8e4
AF = mybir.ActivationFunctionType
OP = mybir.AluOpType
DR = mybir.MatmulPerfMode.DoubleRow

B, S, D, F = 8, 2048, 1024, 3072
EPS = 1e-6
KD = D // 128           # 8 d-ptiles
MD = D // 128           # 8
MFO = F // 128          # 24 f-ptiles
MF2 = 2 * F // 128      # 48 (gate|up)

CH = 512                # s-chunk (both phases)
NCH = S // CH           # 4
NST = CH // 128         # 4 s-tiles per chunk

# fp8 scaling constants (fixed powers of two; inputs are bounded by
# construction: |W{g,v,d}|<=1/32, |W_gate/up|<=1/32, |W_out|<=1/sqrt(3072))
AS = 8.0                # activation quantization scale (normalized acts)
S_MIX = 4096.0          # mixer weight scale      -> |w|*S <= 128
S_GU = 4096.0           # gate weight scale       -> |w|*S <= 128
S_UP = 4.0              # up weight scale (small so z = gate*ups fits fp8)
S_O = 8192.0            # out-proj weight scale   -> |w|*S <= 148
ZETA = AS * S_UP        # scale carried by the fp8 z tile (= 32)


def build_program():
    nc = bacc.Bacc("TRN2", target_bir_lowering=False, debug=False)

    x_d = nc.dram_tensor("x", [S, D], F32R, kind="ExternalInput").ap()
    wmix_d = nc.dram_tensor("w_mix", [3 * MD, 128, KD, 128], F8, kind="ExternalInput").ap()
    bmix_d = nc.dram_tensor("b_mix", [128, 3 * MD], F32, kind="ExternalInput").ap()
    wgu_d = nc.dram_tensor("w_gu", [MF2, 128, KD, 128], F8, kind="ExternalInput").ap()
    wout_d = nc.dram_tensor("w_out", [MD, 128, MFO, 128], F8, kind="ExternalInput").ap()
    ident_d = nc.dram_tensor("ident", [128, 128], F32R, kind="ExternalInput").ap()
    out_d = nc.dram_tensor("out", [S, D], F32, kind="ExternalOutput").ap()

    with tile.TileContext(nc) as tc, ExitStack() as top:
        # ---------- persistent tiles ----------
        cpool = top.enter_context(tc.tile_pool(name="consts", bufs=1))
        ident = cpool.tile([128, 128], F32R)
        nc.sync.dma_start(ident[:], ident_d[:])
        ones_col = cpool.tile([128, 1], F16)
        nc.vector.memset(ones_col[:], 1.0)
        ones8 = cpool.tile([128, 1], F8)
        nc.vector.memset(ones8[:], 1.0)
        bmix = cpool.tile([128, 3 * MD], F32)
        nc.sync.dma_start(bmix[:], bmix_d[:])
        eps1 = cpool.tile([1, 1], F32)
        nc.vector.memset(eps1[:], EPS / (AS * AS))

        # DRAM scratch for the (scaled) mixer output residual, per chunk
        dpool = top.enter_context(tc.tile_pool(name="dscratch", bufs=1, space="DRAM"))
        sc1_t = [dpool.tile([KD, 128, CH], F32, name=f"sc1_{i}") for i in range(NCH)]

        # normalized out1 (x8) stays resident in SBUF across phase 1 -> 2a
        o1n_pool = top.enter_context(tc.tile_pool(name="o1n", bufs=1))
        o1n = o1n_pool.tile([128, KD, S], F8)

        carry_pool = top.enter_context(tc.tile_pool(name="carry", bufs=1))
        carry = carry_pool.tile([128, KD], F32)

        # pools that must survive into phase 2a (last chunk's norm2 is
        # emitted there so its latency chain hides under FFN matmuls)
        p_sq = top.enter_context(tc.tile_pool(name="sq", bufs=2))
        p_o1 = top.enter_context(tc.tile_pool(name="o1", bufs=2))
        p_row = top.enter_context(tc.tile_pool(name="rows", bufs=1))
        p_bc = top.enter_context(tc.tile_pool(name="bc", bufs=2))
        p_wgu = top.enter_context(tc.tile_pool(name="wgu", bufs=6))
        ps_ss = top.enter_context(tc.tile_pool(name="ss_ps", bufs=2, space="PSUM"))

        # ---------- phase 1: mixer ----------
        with ExitStack() as ph1:
            wpool = ph1.enter_context(tc.tile_pool(name="wmix", bufs=1))
            wmix = wpool.tile([128, 3 * MD, KD, 128], F8)
            wmix_dp = wmix_d.rearrange("m p k j -> p m k j")

            p_nat = ph1.enter_context(tc.tile_pool(name="xnat", bufs=5))
            p_xT = ph1.enter_context(tc.tile_pool(name="xT", bufs=2))
            p_x8 = ph1.enter_context(tc.tile_pool(name="x8", bufs=2))
            p_sg = ph1.enter_context(tc.tile_pool(name="sg", bufs=2))
            p_at = ph1.enter_context(tc.tile_pool(name="at", bufs=2))
            p_sm = ph1.enter_context(tc.tile_pool(name="sm", bufs=2))
            p_hT = ph1.enter_context(tc.tile_pool(name="hT", bufs=2))
            ps_tp = ph1.enter_context(tc.tile_pool(name="tp_ps", bufs=2, space="PSUM"))
            ps_mm = ph1.enter_context(tc.tile_pool(name="mm_ps", bufs=4, space="PSUM"))

            st = {}  # c -> dict of live tiles

            def stA(c):
                """load + transpose x chunk; norm1 squares + PE reduce."""
                s0 = c * CH
                d = st.setdefault(c, {})
                xT = p_xT.tile([128, KD, CH], F32, tag="xT", name=f"xT{c}")
                nats = []
                for stt in range(NST):
                    nat = p_nat.tile([128, D], F32R, tag="nat", name=f"nat{c}_{stt}")
                    nc.sync.dma_start(nat[:], x_d[s0 + stt * 128: s0 + (stt + 1) * 128, :])
                    nats.append(nat)
                ss1 = ps_ss.tile([1, CH], F32, tag="ss", name=f"ss1_{c}")
                for kt in range(KD):
                    tp = ps_tp.tile([128, CH], F32R, tag="tp", name=f"tp{c}_{kt}")
                    for stt in range(NST):
                        nc.tensor.transpose(tp[:, stt * 128:(stt + 1) * 128],
                                            nats[stt][:, kt * 128:(kt + 1) * 128],
                                            ident[:])
                    nc.vector.tensor_copy(xT[:, kt], tp[:])
                    if kt % 2 == 1:   # paired fp8 squares + DoubleRow reduce
                        sq = p_sq.tile([128, 2, CH], F8, tag="sq1",
                                       name=f"sq1_{c}_{kt // 2}")
                        nc.scalar.activation(sq[:], xT[:, kt - 1:kt + 1],
                                             AF.Square, bias=0.0)
                        for h in range(2):
                            nc.tensor.matmul(ss1[:], ones8[:], sq[:, h],
                                             start=(kt == 1 and h == 0),
                                             stop=(kt == KD - 1 and h == 1))
                d["xT"] = xT
                d["ss1"] = ss1

            def stB(c):
                """norm1 scale; xnT = AS * x / rms -> fp8."""
                d = st[c]
                srow = p_row.tile([1, CH], F32, tag="srow1", name=f"srow1_{c}")
                nc.scalar.activation(srow[:], d["ss1"][:], AF.Sqrt,
                                     bias=eps1[:], scale=1.0 / (AS * AS * D))
                rrow = p_row.tile([1, CH], F32, tag="rrow1", name=f"rrow1_{c}")
                nc.vector.reciprocal_approx_fast(rrow[:], srow[:])
                bc = p_bc.tile([128, CH], F32, tag="bc1", name=f"bc1_{c}")
                nc.gpsimd.partition_broadcast(bc[:], rrow[:])
                xnT = p_x8.tile([128, KD, CH], F8, tag="xnT", name=f"xnT{c}")
                for kt in range(KD):
                    nc.vector.tensor_tensor(xnT[:, kt], d["xT"][:, kt], bc[:], OP.mult)
                d["xnT"] = xnT

            def _proj(d, mt, out_ap, fn):
                ps = ps_mm.tile([128, CH], F32, tag="mm", name=f"mm_{mt}")
                for j in range(KD // 2):
                    nc.tensor.matmul(ps[:], wmix[:, mt, 2 * j:2 * j + 2, :],
                                     d["xnT"][:, 2 * j:2 * j + 2, :],
                                     start=(j == 0), stop=(j == KD // 2 - 1),
                                     perf_mode=DR)
                nc.scalar.activation(out_ap, ps[:], fn,
                                     bias=bmix[:, mt:mt + 1], scale=1.0 / (AS * S_MIX))

            def stC(c):
                """projections (fp8 DoubleRow), activations, scan, residual."""
                d = st[c]
                sg = p_sg.tile([128, KD, CH], F16, tag="sg", name=f"sg{c}")
                for kt in range(KD):          # g-pass (sigmoid table)
                    _proj(d, kt, sg[:, kt], AF.Sigmoid)
                a_t = p_at.tile([128, KD, CH], F16, tag="at", name=f"at{c}")
                for kt in range(KD):          # d-pass (sigmoid table)
                    sd = p_sm.tile([128, CH], F16, tag="sd", name=f"sd{c}_{kt}")
                    _proj(d, 2 * MD + kt, sd[:], AF.Sigmoid)
                    nc.vector.tensor_scalar(a_t[:, kt], sd[:], 0.998, 0.001,
                                            OP.mult, OP.add)
                out1 = p_o1.tile([128, KD, CH], F32, tag="o1", name=f"o1_{c}")
                for kt in range(KD):          # v-pass (tanh table) + scan chain
                    tv = p_sm.tile([128, CH], F16, tag="tv", name=f"tv{c}_{kt}")
                    _proj(d, MD + kt, tv[:], AF.Tanh)
                    xs = p_sm.tile([128, CH], F16, tag="xs", name=f"xs{c}_{kt}")
                    nc.vector.tensor_tensor(xs[:], sg[:, kt], tv[:], OP.mult)
                    hT = p_hT.tile([128, CH], F32, tag="hT", name=f"hT{c}_{kt}")
                    init = 0.0 if c == 0 else carry[:, kt:kt + 1]
                    nc.vector.tensor_tensor_scan(hT[:], a_t[:, kt], xs[:],
                                                 init, OP.mult, OP.add)
                    nc.vector.tensor_copy(carry[:, kt:kt + 1], hT[:, CH - 1:CH])
                    nc.vector.tensor_tensor(out1[:, kt], d["xT"][:, kt], hT[:],
                                            OP.add)
                    nc.sync.dma_start(sc1_t[c][kt], out1[:, kt])
                d["out1"] = out1
                if c < NCH - 1:
                    stNorm2(c)

            def stNorm2(c):
                """norm2 squares + PE reduce (deferred to 2a for the last
                chunk so the scan-chain drain hides under FFN matmuls)."""
                d = st[c]
                ss2 = ps_ss.tile([1, CH], F32, tag="ss", name=f"ss2_{c}")
                for kh in range(KD // 2):     # paired fp8 squares + DR reduce
                    sq = p_sq.tile([128, 2, CH], F8, tag="sq2",
                                   name=f"sq2_{c}_{kh}")
                    nc.scalar.activation(sq[:], d["out1"][:, 2 * kh:2 * kh + 2],
                                         AF.Square, bias=0.0)
                    for h in range(2):
                        nc.tensor.matmul(ss2[:], ones8[:], sq[:, h],
                                         start=(kh == 0 and h == 0),
                                         stop=(kh == KD // 2 - 1 and h == 1))
                d["ss2"] = ss2

            def stD(c):
                """norm2 scale; o1n = AS * out1 / rms -> fp8 resident."""
                d = st[c]
                s0 = c * CH
                srow = p_row.tile([1, CH], F32, tag="srow2", name=f"srow2_{c}")
                nc.scalar.activation(srow[:], d["ss2"][:], AF.Sqrt,
                                     bias=eps1[:], scale=1.0 / (AS * AS * D))
                rrow = p_row.tile([1, CH], F32, tag="rrow2", name=f"rrow2_{c}")
                nc.vector.reciprocal_approx_fast(rrow[:], srow[:])
                bc = p_bc.tile([128, CH], F32, tag="bc2", name=f"bc2_{c}")
                nc.gpsimd.partition_broadcast(bc[:], rrow[:])
                for kt in range(KD):
                    nc.vector.tensor_tensor(o1n[:, kt, s0:s0 + CH],
                                            d["out1"][:, kt], bc[:], OP.mult)
                del st[c]

            stA(0)
            # mixer weights per e-ptile so the first projections start early
            for mt in range(3 * MD):
                nc.sync.dma_start(wmix[:, mt], wmix_dp[:, mt])
            # prefetch the first two FFN weight pairs behind the mixer weights
            wgu_pre = {}
            for mg in range(2):
                wg = p_wgu.tile([128, KD, 128], F8, tag="wgu")
                nc.sync.dma_start(wg[:], wgu_d[mg])
                wu = p_wgu.tile([128, KD, 128], F8, tag="wgu")
                nc.sync.dma_start(wu[:], wgu_d[MFO + mg])
                wgu_pre[mg] = (wg, wu)
            stB(0)
            for c in range(NCH):
                if c + 1 < NCH:
                    stA(c + 1)
                stC(c)
                if c + 1 < NCH:
                    stB(c + 1)
                if c < NCH - 1:
                    stD(c)

        # ---------- phase 2: FFN ----------
        with ExitStack() as ph2:
            zpool = ph2.enter_context(tc.tile_pool(name="zbuf", bufs=1))
            z = zpool.tile([128, MFO, S], F8)
            wopool = ph2.enter_context(tc.tile_pool(name="wout", bufs=1))
            wout = wopool.tile([128, MD, MFO, 128], F8)
            nc.sync.dma_start(wout[:], wout_d.rearrange("m p k j -> p m k j"))

            # 2a: gate/up + z
            with ExitStack() as ph2a:
                p_gt = ph2a.enter_context(tc.tile_pool(name="gt", bufs=3))
                ps_gu = ph2a.enter_context(tc.tile_pool(name="gu_ps", bufs=4, space="PSUM"))

                def gu_group(mg, sc, wg, wu):
                    sl = slice(sc * CH, (sc + 1) * CH)
                    gps = ps_gu.tile([128, CH], F32, tag="gups")
                    for j in range(KD // 2):
                        nc.tensor.matmul(gps[:], wg[:, 2 * j:2 * j + 2, :],
                                         o1n[:, 2 * j:2 * j + 2, sl],
                                         start=(j == 0), stop=(j == KD // 2 - 1),
                                         perf_mode=DR)
                    ups = ps_gu.tile([128, CH], F32, tag="gups")
                    for j in range(KD // 2):
                        nc.tensor.matmul(ups[:], wu[:, 2 * j:2 * j + 2, :],
                                         o1n[:, 2 * j:2 * j + 2, sl],
                                         start=(j == 0), stop=(j == KD // 2 - 1),
                                         perf_mode=DR)
                    gate = p_gt.tile([128, CH], F16, tag="gate")
                    nc.scalar.activation(gate[:], gps[:], AF.Silu,
                                         bias=0.0, scale=1.0 / (AS * S_GU))
                    # z = silu(G) * (AS*S_UP*U): fp8 tile carries ZETA=32
                    nc.vector.tensor_tensor(z[:, mg, sl], gate[:], ups[:],
                                            OP.mult)

                # chunks 0-2 of the first two f-tiles run while the last
                # mixer chunk's norm2/scan chain drains; then its sc=3 slabs
                for mg in range(2):
                    for sc in range(NCH - 1):
                        gu_group(mg, sc, *wgu_pre[mg])
                stNorm2(NCH - 1)
                stD(NCH - 1)
                for mg in range(2):
                    gu_group(mg, NCH - 1, *wgu_pre[mg])
                for mg in range(2, MFO):
                    wg = p_wgu.tile([128, KD, 128], F8, tag="wgu")
                    nc.sync.dma_start(wg[:], wgu_d[mg])
                    wu = p_wgu.tile([128, KD, 128], F8, tag="wgu")
                    nc.sync.dma_start(wu[:], wgu_d[MFO + mg])
                    for sc in range(NCH):
                        gu_group(mg, sc, wg, wu)

            # 2b: W_out + residual + transpose out
            with ExitStack() as ph2b:
                p_o1c = ph2b.enter_context(tc.tile_pool(name="o1c", bufs=3))
                p_yy = ph2b.enter_context(tc.tile_pool(name="yy", bufs=2))
                p_oT = ph2b.enter_context(tc.tile_pool(name="outT", bufs=MD + 1))
                p_onat = ph2b.enter_context(tc.tile_pool(name="onat", bufs=3))
                ps_y = ph2b.enter_context(tc.tile_pool(name="y_ps", bufs=2, space="PSUM"))
                ps_t2 = ph2b.enter_context(tc.tile_pool(name="t2_ps", bufs=2, space="PSUM"))
                for sc in range(NCH):
                    sl = slice(sc * CH, (sc + 1) * CH)
                    outTs = []
                    for mo in range(MD):
                        yps = ps_y.tile([128, CH], F32, tag="yps")
                        for j in range(MFO // 2):
                            nc.tensor.matmul(yps[:], wout[:, mo, 2 * j:2 * j + 2, :],
                                             z[:, 2 * j:2 * j + 2, sl],
                                             start=(j == 0), stop=(j == MFO // 2 - 1),
                                             perf_mode=DR)
                        o1c = p_o1c.tile([128, CH], F32, tag="o1c")
                        nc.sync.dma_start(o1c[:], sc1_t[sc][mo])
                        yy = p_yy.tile([128, CH], F32, tag="yy")
                        nc.vector.tensor_scalar(yy[:], yps[:], 1.0 / (ZETA * S_O),
                                                0.0, OP.mult, OP.add)
                        oT = p_oT.tile([128, CH], F32R, tag="oT")
                        nc.vector.tensor_tensor(oT[:], yy[:], o1c[:], OP.add)
                        outTs.append(oT)
                    for q in range(NST):
                        onat = p_onat.tile([128, D], F32, tag="onat")
                        for h in range(2):
                            t2 = ps_t2.tile([128, 512], F32R, tag="t2")
                            for jj in range(4):
                                nc.tensor.transpose(
                                    t2[:, jj * 128:(jj + 1) * 128],
                                    outTs[4 * h + jj][:, q * 128:(q + 1) * 128],
                                    ident[:])
                            nc.vector.tensor_copy(onat[:, h * 512:(h + 1) * 512],
                                                  t2[:])
                        srow0 = sc * CH + q * 128
                        nc.sync.dma_start(out_d[srow0:srow0 + 128, :], onat[:])

    nc.compile()
    return nc


_NC = None


def _get_nc():
    global _NC
    if _NC is None:
        _NC = build_program()
    return _NC


def _q8(a, s):
    return np.clip(np.asarray(a, np.float32) * s, -240.0, 240.0).astype(
        ml_dtypes.float8_e4m3)


def _prep_weights(inputs):
    w1 = np.asarray(inputs["rms_mix_w"], np.float32)
    w2 = np.asarray(inputs["rms_ffn_w"], np.float32)
    Wg = np.asarray(inputs["Wg"], np.float32) * w1[None, :]
    Wv = np.asarray(inputs["Wv"], np.float32) * w1[None, :]
    Wd = np.asarray(inputs["Wd"], np.float32) * w1[None, :]
    Wcat = np.concatenate([Wg, Wv, Wd], axis=0)            # [3D, D]
    w_mix = _q8(np.ascontiguousarray(
        Wcat.T.reshape(KD, 128, 3 * MD, 128).transpose(2, 1, 0, 3)), S_MIX)
    bcat = np.concatenate([np.asarray(inputs["bg"], np.float32),
                           np.asarray(inputs["bv"], np.float32),
                           np.asarray(inputs["bd"], np.float32)])
    b_mix = np.ascontiguousarray(bcat.reshape(3 * MD, 128).T).astype(np.float32)
    Wgate = np.asarray(inputs["W_gate"], np.float32) * w2[None, :]
    Wup = np.asarray(inputs["W_up"], np.float32) * w2[None, :]
    Wcat2 = np.concatenate([Wgate * S_GU, Wup * S_UP], axis=0)  # [2F, D], pre-scaled
    w_gu = _q8(np.ascontiguousarray(
        Wcat2.T.reshape(KD, 128, MF2, 128).transpose(2, 1, 0, 3)), 1.0)
    WoT = np.asarray(inputs["W_out"], np.float32).T        # [F, D]
    w_out = _q8(np.ascontiguousarray(
        WoT.reshape(MFO, 128, MD, 128).transpose(2, 1, 0, 3)), S_O)
    return {
        "w_mix": w_mix, "b_mix": b_mix, "w_gu": w_gu, "w_out": w_out,
        "ident": np.eye(128, dtype=np.float32),
    }


def run(inputs, trace=False, **kw):
    x = np.asarray(inputs["x"], np.float32)
    shared = _prep_weights(inputs)
    in_maps = [dict(shared, x=np.ascontiguousarray(x[b])) for b in range(B)]
    res = run_bass_kernel_spmd(_get_nc(), in_maps, list(range(B)), trace=trace, **kw)
    out = np.stack([np.asarray(res.results[b]["out"], np.float32) for b in range(B)])
    return out, res


def kernel(**inputs) -> np.ndarray:
    out, _ = run(inputs)
    return out
